# revision 26
# baseline (speedup 1.0000x reference)
"""AttentionBlock SPMD kernel for 8 TRN2 NeuronCores (v2: bf16 datapath).

Math (matching the reference):
  qkv = x @ W_qkv + b_qkv -> q,k,v per (b, h)
  scores = q k^T / sqrt(64) + bias[h];  attn = softmax(scores)
  out = (attn @ v  concat heads) @ W_proj + b_proj

Sharding: 48 units of (head h, query block qb of 512 rows), batch kept
together per unit so each bias slice is read once fleet-wide. Core c owns
head A=c (4 blocks) and head B=8+c//2 (2 blocks; odd cores work in a
column-permuted coordinate system so the program is SPMD-uniform).

v2 datapath:
  * all matmul operands bf16 (x, weights, q/k/v, attn) -> half the DMA
    bytes of the fp32 baseline at the same PE rate.
  * the additive bias is applied multiplicatively: the host ships
    ebias = exp(bias^T) in bf16 and the kernel computes
    exp(s)*ebias on DVE (fast all-SBUF bf16 multiply), removing the fp32
    PSUM bias-add. The key bias b_k is dropped entirely: q.(k+b_k) adds a
    per-query constant to every key's score, which softmax cancels.
  * exp runs on Act over 2-key-chunk PSUM tiles (1024 elems/instr) to
    amortize per-instruction overhead; scores use a 3-deep 2-bank PSUM
    pool so exps stream back-to-back. The 2 extra banks come from
    processing each unit as two batch pairs (2 live AV accumulators,
    not 4); each unit's 4 bias tiles persist in SBUF so the bias is
    still read only once.
  * phase 1 is emitted query-block-major with unit 0 / batch pair 0
    interleaved, so softmax work starts ~15us in rather than after the
    whole qkv projection.
  * softmax denominator falls out of a ones-column in v; 1/den via
    SBUF-staged reciprocal_approx_fast (DVE; the custom op misreads
    PSUM operands) + partition_broadcast (Pool). Normalization runs
    inline right after each batch's last AV chunk.
  * output partials are written as fp16 (half the DMA), host-summed in
    fp32. Projection matmuls are spread through the following unit's
    inner loop (PSUM borrowed from the score pool, 1 task per 4
    iterations); the remainder and the final query block drain at the
    tail via wide psum tiles, copies alternating DVE/Act, and both DMA
    generators.
  * per-unit bias tiles are prefetched at unit start (SWDGE drains them
    over the unit's ~30us); x tiles prefetch 4 deep.
  * GPSIMD never touches PSUM (hardware restriction).
"""

import numpy as np

B, N, D, H, HD = 4, 2048, 768, 12, 64
R = B * N                # 8192 flattened rows
P = 128                  # partitions
FD = 512                 # free-dim tile (query block)
NKD = D // P             # 6 contraction chunks over D
NRT = R // FD            # 16 row tiles
NKC = N // P             # 16 key chunks per batch
NCORES = 8
SCALE = 1.0 / np.sqrt(HD)

# phase-2 unit order: B-head units early so projection of the covered
# query blocks can start mid-phase.  (slot, qb): slot 1 = head B.
ULIST = [(1, 0), (0, 0), (1, 1), (0, 1), (0, 2), (0, 3)]
# after unit index ui completes, these query blocks are fully computed
PROJ_AFTER = {0: [], 1: [0], 2: [], 3: [1], 4: [2], 5: [3]}

_NC = None               # compiled module cache
TRACE = False
LAST_PROFILE = None


def _build_module(reps=1):
    import concourse.bacc as bacc
    import concourse.tile as tile
    from concourse import mybir
    from concourse.masks import make_identity

    f32 = mybir.dt.float32
    f32r = mybir.dt.float32r
    bf16 = mybir.dt.bfloat16
    f16 = mybir.dt.float16
    Ident = mybir.ActivationFunctionType.Identity
    Exp = mybir.ActivationFunctionType.Exp

    nc = bacc.Bacc("TRN2", target_bir_lowering=False, debug=False,
                   num_devices=NCORES)

    xT_d = nc.dram_tensor("xT", [D, R], bf16, kind="ExternalInput").ap()
    wq_d = nc.dram_tensor("wq", [D, P], bf16, kind="ExternalInput").ap()
    wk_d = nc.dram_tensor("wk", [D, P], bf16, kind="ExternalInput").ap()
    wv_d = nc.dram_tensor("wv", [D, P], bf16, kind="ExternalInput").ap()
    bqkv_d = nc.dram_tensor("bqkv", [P, 3], f32, kind="ExternalInput").ap()
    ebiasT_d = nc.dram_tensor("ebiasT", [6, N, FD], bf16,
                              kind="ExternalInput").ap()
    wproj_d = nc.dram_tensor("wproj", [P, D], bf16, kind="ExternalInput").ap()
    outT_d = nc.dram_tensor("outT", [D, R], f16, kind="ExternalOutput").ap()

    xT_r = xT_d.rearrange("(kc p) r -> p kc r", p=P)          # (128, 6, 8192)
    outT_r = outT_d.rearrange("(mc p) r -> p mc r", p=P)      # (128, 6, 8192)

    with tile.TileContext(nc) as tc:
        with (
            tc.tile_pool(name="const", bufs=1) as const,
            tc.tile_pool(name="persist", bufs=1) as persist,
            tc.tile_pool(name="psum_mm", bufs=3, space="PSUM") as psmm,
            tc.tile_pool(name="psum_av", bufs=2, space="PSUM") as psav,
            tc.tile_pool(name="xt", bufs=4) as xtp,
            tc.tile_pool(name="wk1", bufs=2) as wk1,
            tc.tile_pool(name="bias", bufs=9) as biasp,
            tc.tile_pool(name="eq", bufs=7) as eqp,
            tc.tile_pool(name="eqb", bufs=7) as eqbp,
            tc.tile_pool(name="small", bufs=2) as small,
            tc.tile_pool(name="ot", bufs=8) as otp,
        ):
            # ---- constants ----
            # wv + the first x tile are the startup critical path: keep them
            # alone on the HWDGE generator; route the other constants through
            # SWDGE (Pool) so their descriptor generation doesn't delay xt0
            wq_t = const.tile([P, NKD, P], bf16)
            wk_t = const.tile([P, NKD, P], bf16)
            wv_t = const.tile([P, NKD, P], bf16)
            nc.sync.dma_start(out=wv_t,
                              in_=wv_d.rearrange("(kc p) m -> p kc m", p=P))
            for w_t, w_d in ((wq_t, wq_d), (wk_t, wk_d)):
                nc.gpsimd.dma_start(
                    out=w_t, in_=w_d.rearrange("(kc p) m -> p kc m", p=P))
            bqkv_t = const.tile([P, 3], f32)
            nc.gpsimd.dma_start(out=bqkv_t, in_=bqkv_d)
            # wproj is first needed mid-phase-2; load off the startup path
            wproj_t = const.tile([P, D], bf16)
            nc.gpsimd.dma_start(out=wproj_t, in_=wproj_d)
            ident = const.tile([P, P], bf16)
            make_identity(nc, ident)
            ones128 = const.tile([P, 2 * NKC], bf16)
            nc.vector.memset(ones128, 1.0)
            # dummy activation at t=0 so the Exp/Identity act-table load
            # happens during startup instead of delaying the first real
            # activation mid-phase-1
            warm = const.tile([1, 16], f32)
            nc.scalar.activation(warm, ones128[0:1, 0:16],
                                 mybir.ActivationFunctionType.Exp)
            # tiny PE warm-up on an operand ready at t~0 (ones128, unlike
            # ident which waits behind Pool's SWDGE triggers) so the PE
            # p-state ramp starts before the first x tile arrives
            warm_ps = psmm.tile([P, 2, FD], f32, tag="mm")
            for _w in range(3):
                nc.tensor.matmul(warm_ps[0:32, 0, 0:32],
                                 ones128[:, 0:32], ones128[:, 0:32],
                                 start=True, stop=True)

            # ---- persistent buffers (per-b granularity for overlap) ----
            # partition halves: rows 0-63 = head A, 64-127 = head B
            qTb = [persist.tile([P, N], bf16, name=f"qT{b}") for b in range(B)]
            kTb = [persist.tile([P, N], bf16, name=f"kT{b}") for b in range(B)]
            # v keys-major: [key_in_chunk, slot, kc, hd(64)+ones(1)]
            vb = [persist.tile([P, 2, NKC, HD + 1], bf16, name=f"v{b}")
                  for b in range(B)]
            # attn^T per (qb, b): rows = 2 head slots
            atb = [[persist.tile([P, FD], bf16, name=f"at{qb}_{b}")
                    for b in range(B)] for qb in range(4)]
            for b in range(B):
                nc.vector.tensor_copy(
                    out=vb[b][:, :, :, HD:HD + 1],
                    in_=ones128.rearrange("p (a c) -> p a c", a=2)[:, :, :, None])

            for _rep in range(reps):
                    pending = []   # deferred projection tasks (pqb, b, mcpair)

                    def emit_rt(b_i, qb):
                        """qkv projection for row tile (b_i, qb): v first so
                        the transpose chain overlaps the q/k gemms."""
                        rt = b_i * 4 + qb
                        cols = slice(rt * FD, (rt + 1) * FD)
                        qcols = slice(qb * FD, (qb + 1) * FD)
                        xt = xtp.tile([P, NKD, FD], bf16, tag="xt")
                        eng = nc.sync if rt % 2 == 0 else nc.gpsimd
                        eng.dma_start(out=xt, in_=xT_r[:, :, cols])
                        ps_v = psmm.tile([P, 2, FD], f32, tag="mm")
                        for kc in range(NKD):
                            nc.tensor.matmul(ps_v[:, 0, :], wv_t[:, kc, :],
                                             xt[:, kc, :],
                                             start=(kc == 0),
                                             stop=(kc == NKD - 1))
                        vt_sb = wk1.tile([P, FD], bf16, tag="vt")
                        nc.scalar.activation(vt_sb, ps_v[:, 0, :], Ident,
                                             bias=bqkv_t[:, 2:3])
                        # q into bank 0, k into bank 1 of one psum tile
                        ps_qk = psmm.tile([P, 2, FD], f32, tag="mm")
                        for g in range(2):
                            w_t = (wq_t, wk_t)[g]
                            for kc in range(NKD):
                                nc.tensor.matmul(ps_qk[:, g, :], w_t[:, kc, :],
                                                 xt[:, kc, :],
                                                 start=(kc == 0),
                                                 stop=(kc == NKD - 1))
                        tp = psmm.tile([P, 4, P], bf16, tag="mm")
                        for j in range(4):
                            nc.tensor.transpose(
                                tp[:, j, :], vt_sb[:, j * P:(j + 1) * P],
                                ident)
                        nc.vector.tensor_copy(
                            out=vb[b_i][:, :, qb * 4:(qb + 1) * 4, 0:HD],
                            in_=tp.rearrange("p j (s hd) -> p s j hd", s=2))
                        if qb < 2:
                            nc.scalar.activation(
                                qTb[b_i][:, qcols], ps_qk[:, 0, :], Ident,
                                bias=bqkv_t[:, 0:1])
                        else:
                            nc.scalar.activation(
                                qTb[b_i][0:64, qcols], ps_qk[0:64, 0, :],
                                Ident, bias=bqkv_t[0:64, 0:1])
                        # k bias dropped (softmax-invariant): plain copy
                        nc.vector.tensor_copy(kTb[b_i][:, qcols],
                                              ps_qk[:, 1, :])

                    def emit_proj(task):
                        pqb, b_i, mp = task
                        covered = pqb < 2
                        cols = slice(b_i * N + pqb * FD,
                                     b_i * N + (pqb + 1) * FD)
                        ps = psmm.tile([P, 2, FD], f32, tag="mm")
                        for hm in range(2):
                            mc = mp * 2 + hm
                            msl = slice(mc * P, (mc + 1) * P)
                            if covered:
                                nc.tensor.matmul(ps[:, hm, :],
                                                 wproj_t[:, msl],
                                                 atb[pqb][b_i],
                                                 start=True, stop=True)
                            else:
                                nc.tensor.matmul(ps[:, hm, :],
                                                 wproj_t[0:64, msl],
                                                 atb[pqb][b_i][0:64, :],
                                                 start=True, stop=True)
                        ot = otp.tile([P, 2, FD], f16, tag="ot")
                        # GPSIMD cannot read PSUM; DVE has the most slack
                        nc.vector.tensor_copy(ot, ps)
                        nc.sync.dma_start(
                            out=outT_r[:, mp * 2:mp * 2 + 2, cols], in_=ot)

                    def emit_proj_tail(task, idx, deng=None):
                        # tail-drain variant: single-mc psum tiles spread
                        # across both psum pools, copies fanned over
                        # DVE/Act/Pool (all idle at the tail), DMAs split
                        # between HWDGE and SWDGE
                        pqb, b_i, mp = task
                        covered = pqb < 2
                        cols = slice(b_i * N + pqb * FD,
                                     b_i * N + (pqb + 1) * FD)
                        ot = otp.tile([P, 2, FD], f16, tag="ot")
                        ps = psmm.tile([P, 2, FD], f32, tag="mm")
                        for hm in range(2):
                            mc = mp * 2 + hm
                            msl = slice(mc * P, (mc + 1) * P)
                            if covered:
                                nc.tensor.matmul(ps[:, hm, :], wproj_t[:, msl],
                                                 atb[pqb][b_i],
                                                 start=True, stop=True)
                            else:
                                nc.tensor.matmul(ps[:, hm, :],
                                                 wproj_t[0:64, msl],
                                                 atb[pqb][b_i][0:64, :],
                                                 start=True, stop=True)
                        # one wide copy per task, alternating engines for
                        # 2-engine throughput (Pool cannot read PSUM)
                        if idx % 2 == 0:
                            nc.vector.tensor_copy(ot, ps)
                        else:
                            nc.scalar.copy(ot, ps)
                        # split drain DMAs over both descriptor generators
                        deng = nc.sync if idx % 2 == 0 else nc.gpsimd
                        deng.dma_start(
                            out=outT_r[:, mp * 2:mp * 2 + 2, cols], in_=ot)

                    def emit_proj_boundary(task):
                        # batch-pair-boundary variant: PE-only fill for the
                        # norm-chain WAR hole. The PSUM->SBUF copy runs on
                        # Act (idle here: the next bp's scores aren't in
                        # PSUM yet) so DVE/Pool stay clear for the norm
                        # chain that releases the AV banks.
                        pqb, b_i, mp = task
                        covered = pqb < 2
                        cols = slice(b_i * N + pqb * FD,
                                     b_i * N + (pqb + 1) * FD)
                        ps = psmm.tile([P, 2, FD], f32, tag="mm")
                        for hm in range(2):
                            mc = mp * 2 + hm
                            msl = slice(mc * P, (mc + 1) * P)
                            if covered:
                                nc.tensor.matmul(ps[:, hm, :],
                                                 wproj_t[:, msl],
                                                 atb[pqb][b_i],
                                                 start=True, stop=True)
                            else:
                                nc.tensor.matmul(ps[:, hm, :],
                                                 wproj_t[0:64, msl],
                                                 atb[pqb][b_i][0:64, :],
                                                 start=True, stop=True)
                        ot = otp.tile([P, 2, FD], f16, tag="ot")
                        nc.scalar.copy(ot, ps)
                        nc.sync.dma_start(
                            out=outT_r[:, mp * 2:mp * 2 + 2, cols], in_=ot)

                    def emit_norm_b(slot, qb, av, bi, b_i, tail_proj):
                        pb = slot * 64
                        # custom-DVE recip needs a plain SBUF fp32 operand:
                        # stage the psum denominator row through SBUF first
                        dn = small.tile([1, FD], f32, tag="dn")
                        nc.vector.tensor_copy(dn, av[bi][HD:HD + 1, :])
                        rd = small.tile([1, FD], f32, tag="rd")
                        nc.vector.reciprocal_approx_fast(out=rd, in_=dn)
                        rb = small.tile([64, FD], f32, tag="rb")
                        nc.gpsimd.partition_broadcast(rb, rd)
                        nc.vector.tensor_mul(
                            atb[qb][b_i][pb:pb + 64, :],
                            av[bi][0:HD, :], rb)
                        if tail_proj:
                            for mp in range(NKD // 2):
                                emit_proj((qb, b_i, mp))

                    def emit_iter(ui, slot, qb, bp, quarter, hh, bi, av, bt):
                        b_i = bp * 2 + bi
                        pb = slot * 64
                        qcols = slice(qb * FD, (qb + 1) * FD)
                        it = bp * 16 + quarter * 4 + hh * 2 + bi
                        j0 = quarter * 4 + hh * 2
                        sps = psmm.tile([P, 2, FD], f32, tag="mm")
                        for jj in range(2):
                            kc = j0 + jj
                            kcols = slice(kc * P, (kc + 1) * P)
                            nc.tensor.matmul(
                                sps[:, jj, :],
                                kTb[b_i][pb:pb + 64, kcols],
                                qTb[b_i][pb:pb + 64, qcols],
                                start=True, stop=True)
                        eq = eqp.tile([P, 2, FD], bf16, tag="eq")
                        nc.scalar.activation(eq, sps, Exp)
                        eqb = eqbp.tile([P, 2, FD], bf16, tag="eqb")
                        nc.vector.tensor_mul(
                            eqb, eq, bt[:, hh * 2:hh * 2 + 2, :])
                        for jj in range(2):
                            kc = j0 + jj
                            nc.tensor.matmul(
                                av[bi], vb[b_i][:, slot, kc, :],
                                eqb[:, jj, :],
                                start=(kc == 0),
                                stop=(kc == NKC - 1))
                        # spread projection thinly (1 task per 4
                        # iterations, DVE-copy budget bound); the leftovers
                        # drain efficiently through the dual-pool tail
                        if pending and it % 4 == 2:
                            emit_proj(pending.pop(0))
                        # normalize each batch right after its last
                        # AV accumulation so its psum slot frees early
                        if quarter == 3 and hh == 1:
                            emit_norm_b(slot, qb, av, bi, b_i, False)

                    def load_bias(ui, quarter):
                        bt = biasp.tile([P, 4, FD], bf16, tag="bt")
                        nc.gpsimd.dma_start(
                            out=bt,
                            in_=ebiasT_d[ui].rearrange(
                                "(kc p) q -> p kc q", p=P)[
                                :, quarter * 4:(quarter + 1) * 4, :])
                        return bt

                    def emit_bp_quarter(ui, slot, qb, bp, quarter, av, bt):
                        """One quarter (4 key chunks) for batch pair bp.
                        bt is the unit's bias tile for this quarter, loaded
                        once and shared by both batch pairs."""
                        for bi in range(2):
                            for hh in range(2):
                                emit_iter(ui, slot, qb, bp, quarter, hh, bi,
                                          av, bt)

                    # ---- phase 1 (qb-major) with unit 0 / batch pair 0
                    # interleaved: after the (b0, b1) row tiles of each query
                    # block, unit 0's scores for that key quarter can run ----
                    slot0, qb0 = ULIST[0]
                    av0 = [psav.tile([HD + 1, FD], f32, tag="av",
                                     name=f"av_u0p0b{bb}") for bb in range(2)]
                    bts0 = []
                    for qb_g in range(4):
                        bts0.append(load_bias(0, qb_g))
                        for b_i in (0, 1):
                            emit_rt(b_i, qb_g)
                        emit_bp_quarter(0, slot0, qb0, 0, qb_g, av0, bts0[qb_g])
                        for b_i in (2, 3):
                            emit_rt(b_i, qb_g)

                    # ---- phase 2: remaining batch pairs / units ----
                    av01 = [psav.tile([HD + 1, FD], f32, tag="av",
                                      name=f"av_u0p1b{bb}") for bb in range(2)]
                    for quarter in range(4):
                        emit_bp_quarter(0, slot0, qb0, 1, quarter, av01,
                                        bts0[quarter])
                    for ui, (slot, qb) in enumerate(ULIST):
                        if ui == 0:
                            continue
                        # bias loads stay per-quarter: issuing all four as a
                        # burst at unit start occupies the Pool queue exactly
                        # when the norm chain needs partition_broadcast,
                        # which can trigger a long HAM half-clock stretch
                        bts = []
                        for bp in range(2):
                            av = [psav.tile([HD + 1, FD], f32, tag="av",
                                            name=f"av_u{ui}p{bp}b{bb}")
                                  for bb in range(2)]
                            for quarter in range(4):
                                if bp == 0:
                                    bts.append(load_bias(ui, quarter))
                                emit_bp_quarter(ui, slot, qb, bp, quarter, av,
                                                bts[quarter])
                            # this bp's atb slices are final after its
                            # norms: release their projection tasks half a
                            # unit earlier than unit-end (head-B rows for
                            # qb<2 come from units that finished earlier;
                            # qb>=2 is head-A-only)
                            for pqb in PROJ_AFTER[ui]:
                                for b_i in (2 * bp, 2 * bp + 1):
                                    for mp in range(NKD // 2):
                                        pending.append((pqb, b_i, mp))
                            # batch-pair boundary: the next bp's first AV
                            # matmuls WAR-wait on the norm chain releasing
                            # the AV psum banks; feed the PE proj work with
                            # Act-side copies so the HAM clock-gate holds
                            for _ in range(2):
                                if pending:
                                    emit_proj_boundary(pending.pop(0))
                    for idx, task in enumerate(pending):
                        emit_proj_tail(task, idx)
                    pending.clear()

    nc.compile()
    return nc


def _get_module():
    global _NC
    if _NC is None:
        _NC = _build_module()
    return _NC


def _host_pack(x, bias, W_qkv, b_qkv, W_proj):
    """Build the 8 per-core input maps."""
    import ml_dtypes
    bf16 = ml_dtypes.bfloat16

    x = np.asarray(x, dtype=np.float32)
    bias = np.asarray(bias, dtype=np.float32)
    W_qkv = np.asarray(W_qkv, dtype=np.float32)
    b_qkv = np.asarray(b_qkv, dtype=np.float32)
    W_proj = np.asarray(W_proj, dtype=np.float32)

    xT = np.ascontiguousarray(x.reshape(R, D).T).astype(bf16)  # (768, 8192)
    # odd cores: n -> (n + 1024) % 2048 within each batch
    xT_odd = np.ascontiguousarray(
        xT.reshape(D, B, 2, N // 2)[:, :, ::-1, :].reshape(D, R))

    in_maps = []
    for c in range(NCORES):
        hA, hB, halfB = c, 8 + c // 2, c % 2
        odd = halfB == 1

        def wcols(which, h):
            return W_qkv[:, which * D + h * HD: which * D + (h + 1) * HD]

        wq = np.concatenate([wcols(0, hA), wcols(0, hB)], axis=1) * SCALE
        wk = np.concatenate([wcols(1, hA), wcols(1, hB)], axis=1)
        wv = np.concatenate([wcols(2, hA), wcols(2, hB)], axis=1)
        bq = np.concatenate([b_qkv[hA * HD:(hA + 1) * HD],
                             b_qkv[hB * HD:(hB + 1) * HD]]) * SCALE
        bk = np.zeros(P, dtype=np.float32)   # k bias cancels in softmax
        bv = np.concatenate([b_qkv[2 * D + hA * HD: 2 * D + (hA + 1) * HD],
                             b_qkv[2 * D + hB * HD: 2 * D + (hB + 1) * HD]])
        bqkv = np.stack([bq, bk, bv], axis=1)                  # (128, 3)

        def head_bias(h):
            hb = bias[0, h]                                    # (q, k) true coords
            if odd:
                hb = np.roll(np.roll(hb, -N // 2, axis=0), -N // 2, axis=1)
            return hb

        bA, bB = head_bias(hA), head_bias(hB)
        ebiasT = np.empty((6, N, FD), dtype=bf16)
        for ui, (slot, qb) in enumerate(ULIST):
            h_m = bB if slot == 1 else bA
            ebiasT[ui] = np.exp(h_m[qb * FD:(qb + 1) * FD, :].T).astype(bf16)

        wproj = np.concatenate([W_proj[hA * HD:(hA + 1) * HD, :],
                                W_proj[hB * HD:(hB + 1) * HD, :]], axis=0)

        in_maps.append({
            "xT": xT_odd if odd else xT,
            "wq": np.ascontiguousarray(wq.astype(bf16)),
            "wk": np.ascontiguousarray(wk.astype(bf16)),
            "wv": np.ascontiguousarray(wv.astype(bf16)),
            "bqkv": np.ascontiguousarray(bqkv),
            "ebiasT": ebiasT,
            "wproj": np.ascontiguousarray(wproj.astype(bf16)),
        })
    return in_maps


def kernel(x, bias, W_qkv, b_qkv, W_proj, b_proj):
    global LAST_PROFILE
    from concourse.bass_utils import run_bass_kernel_spmd

    nc = _get_module()
    in_maps = _host_pack(x, bias, W_qkv, b_qkv, W_proj)
    res = run_bass_kernel_spmd(nc, in_maps, list(range(NCORES)),
                               trace=TRACE)
    LAST_PROFILE = res
    outT = np.zeros((D, R), dtype=np.float32)
    for c in range(NCORES):
        part = np.asarray(res.results[c]["outT"], dtype=np.float32)
        if c % 2 == 1:  # undo column permutation (involution)
            part = part.reshape(D, B, 2, N // 2)[:, :, ::-1, :].reshape(D, R)
        outT += part
    out = outT.T + np.asarray(b_proj, dtype=np.float32)
    return out.reshape(B, N, D)

